# revision 8
# baseline (speedup 1.0000x reference)
"""DLRM forward on 8 Trainium2 NeuronCores (Bass/Tile), data-parallel over batch.

Strategy: replicate embedding tables (bf16) and all MLP weights on every core;
each core handles B/8 = 2048 samples end to end (no collectives).
  - Embedding bag gather: custom dma_gather (SWDGE) with signed int16 indices
    biased to the table midpoint (reach +-32768 rows covers 50000), one
    descriptor per lookup, 4 SWDGE queues round-robin, trailing pad chunk of
    idx 0 so the index list never ends with a negative index.
  - Pooling + transpose fused into PE matmuls: out = G_chunk.T @ SEL with a
    fixed selection matrix (slot p -> sample p//4) -> feature-major pooled.
  - Per-sample 27x27 Gram matmuls on PE; full (symmetrized, zero-diagonal)
    Z fed to the top MLP's first layer so no tril extraction is needed.
  - MLPs in bf16 with fp32 PSUM accumulation; ReLU/Sigmoid + bias on ScalarE.
"""
import numpy as np
import ml_dtypes

import concourse.bacc as bacc
import concourse.mybir as mybir
from concourse.tile import TileContext
from concourse import bass_utils

BF = mybir.dt.bfloat16
F32 = mybir.dt.float32
I16 = mybir.dt.int16

B = 16384
BL = 2048            # samples per core
T = 26
D = 128
L = 4
NROWS = 50000
BASE = 25000         # index bias -> signed int16 offsets
NBLK = 4             # sample blocks per core
SB = 512             # samples per block
REAL = SB * L        # real gather slots per (table, block)
PAD = 128            # pad slots (idx 0) so the list ends non-negative
NUM = REAL + PAD     # 2176 indices per dma_gather
IW = NUM // 16       # idx columns per instruction (136)
NF = T + 1           # 27 interaction features; feature 26 = bottom-MLP x

_cached = {}


def _build_program():
    nc = bacc.Bacc(dynamic_dma_scratch_size=49152, num_swdge_queues=4)

    tbls = nc.dram_tensor("tbls", [T * NROWS, D], BF, kind="ExternalInput")
    idx = nc.dram_tensor("idx", [128, NBLK * T * IW], I16, kind="ExternalInput")
    xt = nc.dram_tensor("xt", [13, BL], BF, kind="ExternalInput")
    sel = nc.dram_tensor("sel", [128, 32], BF, kind="ExternalInput")
    w1b = nc.dram_tensor("w1b", [13, 512], BF, kind="ExternalInput")
    w2b = nc.dram_tensor("w2b", [128, 4 * 256], BF, kind="ExternalInput")
    w3b = nc.dram_tensor("w3b", [128, 2 * 128], BF, kind="ExternalInput")
    bb1 = nc.dram_tensor("bb1", [128, 4], F32, kind="ExternalInput")
    bb2 = nc.dram_tensor("bb2", [128, 2], F32, kind="ExternalInput")
    bb3 = nc.dram_tensor("bb3", [128, 1], F32, kind="ExternalInput")
    w1x = nc.dram_tensor("w1x", [128, 1024], BF, kind="ExternalInput")
    wz = nc.dram_tensor("wz", [128, 8 * 1024], BF, kind="ExternalInput")  # (kt*4+v)
    b1 = nc.dram_tensor("b1", [128, 8], F32, kind="ExternalInput")
    w2 = nc.dram_tensor("w2", [128, 8 * 1024], BF, kind="ExternalInput")
    b2 = nc.dram_tensor("b2", [128, 8], F32, kind="ExternalInput")
    w3 = nc.dram_tensor("w3", [128, 8 * 512], BF, kind="ExternalInput")
    b3 = nc.dram_tensor("b3", [128, 4], F32, kind="ExternalInput")
    w4 = nc.dram_tensor("w4", [128, 4 * 256], BF, kind="ExternalInput")
    b4 = nc.dram_tensor("b4", [128, 2], F32, kind="ExternalInput")
    w5 = nc.dram_tensor("w5", [128, 2 * 1], BF, kind="ExternalInput")
    b5 = nc.dram_tensor("b5", [1, 1], F32, kind="ExternalInput")
    out = nc.dram_tensor("out", [1, BL], F32, kind="ExternalOutput")

    AF = mybir.ActivationFunctionType
    from contextlib import ExitStack
    with TileContext(nc) as tc:
        with (
            tc.tile_pool(name="wpool", bufs=1) as wp,
            tc.tile_pool(name="idxp", bufs=1) as idxp,
            tc.tile_pool(name="gat", bufs=2) as gat,
            tc.tile_pool(name="hseq", bufs=1) as hp,
            tc.tile_pool(name="pspool", bufs=2, space="PSUM") as psp,
            tc.tile_pool(name="psgram", bufs=2, space="PSUM") as psg,
            tc.tile_pool(name="psmlp", bufs=2, space="PSUM") as psm,
        ):
            # ---- load constants / weights ----
            def ld(dram, shape, dtype):
                t_ = wp.tile(shape, dtype, tag=dram.name)
                nc.sync.dma_start(t_[:], dram[:])
                return t_

            sel_sb = ld(sel, [128, 32], BF)
            w1x_sb = ld(w1x, [128, 1024], BF)
            wz_sb = ld(wz, [128, 8 * 1024], BF)
            b1_sb = ld(b1, [128, 8], F32)
            w2_sb = ld(w2, [128, 8 * 1024], BF)
            b2_sb = ld(b2, [128, 8], F32)
            w3_sb = ld(w3, [128, 8 * 512], BF)
            b3_sb = ld(b3, [128, 4], F32)
            w4_sb = ld(w4, [128, 4 * 256], BF)
            b4_sb = ld(b4, [128, 2], F32)
            w5_sb = ld(w5, [128, 2 * 1], BF)
            b5_sb = ld(b5, [1, 1], F32)

            # ---- bottom MLP (feature-major, bf16, fp32 accum) ----
            hbot_ctx = tc.tile_pool(name="hbot", bufs=1)
            hb = hbot_ctx.__enter__()

            def ldb(dram, shape, dtype):
                t_ = hb.tile(shape, dtype, tag=dram.name)
                nc.sync.dma_start(t_[:], dram[:])
                return t_

            xt_sb = ldb(xt, [13, BL], BF)
            w1b_sb = ldb(w1b, [13, 512], BF)
            w2b_sb = ldb(w2b, [128, 4 * 256], BF)
            w3b_sb = ldb(w3b, [128, 2 * 128], BF)
            bb1_sb = ldb(bb1, [128, 4], F32)
            bb2_sb = ldb(bb2, [128, 2], F32)
            bb3_sb = ldb(bb3, [128, 1], F32)
            h1b = hb.tile([128, 4 * BL], BF, tag="h1b")
            for m in range(4):
                for n in range(4):
                    ps = psm.tile([128, 512], F32, tag="psm")
                    nc.tensor.matmul(ps[:], w1b_sb[:, m * 128:(m + 1) * 128],
                                     xt_sb[:, n * 512:(n + 1) * 512],
                                     start=True, stop=True)
                    nc.scalar.activation(
                        h1b[:, m * BL + n * 512:m * BL + (n + 1) * 512], ps[:],
                        AF.Relu, bias=bb1_sb[:, m:m + 1])
            h2b = hb.tile([128, 2 * BL], BF, tag="h2b")
            for m in range(2):
                for n in range(4):
                    ps = psm.tile([128, 512], F32, tag="psm")
                    for kt in range(4):
                        nc.tensor.matmul(
                            ps[:],
                            w2b_sb[:, kt * 256 + m * 128:kt * 256 + (m + 1) * 128],
                            h1b[:, kt * BL + n * 512:kt * BL + (n + 1) * 512],
                            start=(kt == 0), stop=(kt == 3))
                    nc.scalar.activation(
                        h2b[:, m * BL + n * 512:m * BL + (n + 1) * 512], ps[:],
                        AF.Relu, bias=bb2_sb[:, m:m + 1])
            x3t = hp.tile([128, BL], BF, tag="x3t")
            for n in range(4):
                ps = psm.tile([128, 512], F32, tag="psm")
                for kt in range(2):
                    nc.tensor.matmul(
                        ps[:], w3b_sb[:, kt * 128:(kt + 1) * 128],
                        h2b[:, kt * BL + n * 512:kt * BL + (n + 1) * 512],
                        start=(kt == 0), stop=(kt == 1))
                nc.scalar.activation(x3t[:, n * 512:(n + 1) * 512], ps[:],
                                     AF.Relu, bias=bb3_sb[:, 0:1])
            hbot_ctx.__exit__(None, None, None)

            blk_stack = ExitStack()
            fmp = blk_stack.enter_context(tc.tile_pool(name="fm", bufs=2))
            zslp = blk_stack.enter_context(tc.tile_pool(name="zsl", bufs=1))
            zpkp = blk_stack.enter_context(tc.tile_pool(name="zpk", bufs=1))
            actp = blk_stack.enter_context(tc.tile_pool(name="act", bufs=1))
            outp = blk_stack.enter_context(tc.tile_pool(name="outp", bufs=1))

            qn = 0
            for b in range(NBLK):
                idxb = idxp.tile([128, T * IW], I16, tag="idxb")
                nc.sync.dma_start(idxb[:], idx[:, b * T * IW:(b + 1) * T * IW])

                fm = fmp.tile([128, SB * NF], BF, tag="fm")
                fm3 = fm[:].rearrange("p (s f) -> p s f", f=NF)
                # bottom-MLP x into feature slot 26
                nc.vector.tensor_copy(fm3[:, :, T], x3t[:, b * SB:(b + 1) * SB])

                # ---- gathers + pooling (feature-major out) ----
                for t in range(T):
                    g = gat.tile([128, 17, D], BF, tag="g")
                    nc.gpsimd.dma_gather(
                        out_ap=g[:],
                        in_ap=tbls[t * NROWS + BASE:, :],
                        idxs_ap=idxb[:, t * IW:(t + 1) * IW],
                        num_idxs=NUM, num_idxs_reg=NUM, elem_size=D,
                        single_packet=False, queue_num=qn % 4)
                    qn += 1
                    for q in range(4):  # 128-sample groups
                        pp = psp.tile([128, 128], F32, tag="pp")
                        for cl in range(4):
                            nc.tensor.matmul(pp[:, cl * 32:(cl + 1) * 32],
                                             g[:, q * 4 + cl, :], sel_sb[:],
                                             start=True, stop=True)
                        cp = nc.vector if (t + q) % 2 == 0 else nc.scalar
                        if cp is nc.vector:
                            nc.vector.tensor_copy(
                                fm3[:, q * 128:(q + 1) * 128, t], pp[:])
                        else:
                            nc.scalar.activation(
                                fm3[:, q * 128:(q + 1) * 128, t], pp[:], AF.Copy)

                # ---- per-sample Gram matmuls -> Zslab ----
                zsl = zslp.tile([32, 28 * SB], BF, tag="zsl")
                zv = zsl[:].rearrange("p (i s) -> p i s", s=SB)
                for u in range(SB // 16):
                    zp = psg.tile([32, 16 * 27], F32, tag="zp")
                    for w_ in range(16):
                        s = u * 16 + w_
                        nc.tensor.matmul(zp[0:27, w_ * 27:(w_ + 1) * 27],
                                         fm3[:, s, :], fm3[:, s, :],
                                         start=True, stop=True)
                    src = zp[:].rearrange("p (u i) -> p i u", i=27)[0:27, :, :]
                    nc.vector.tensor_copy(zv[0:27, 0:27, u * 16:(u + 1) * 16], src)

                # ---- pack Z for K=128 layer-1 contraction ----
                zp1 = zpkp.tile([128, 4 * SB], BF, tag="zp1")
                zp2 = zpkp.tile([128, 4 * SB], BF, tag="zp2")
                nc.any.memset(zp1[:], 0.0)
                nc.any.memset(zp2[:], 0.0)
                for g_ in range(4):
                    nc.sync.dma_start(zp1[32 * g_:32 * g_ + 27, :],
                                      zsl[0:27, 4 * g_ * SB:(4 * g_ + 4) * SB])
                for g_ in range(3):
                    w_cols = 4 * SB if g_ < 2 else 3 * SB
                    nc.sync.dma_start(
                        zp2[32 * g_:32 * g_ + 27, 0:w_cols],
                        zsl[0:27, (16 + 4 * g_) * SB:(16 + 4 * g_) * SB + w_cols])

                # ---- top MLP layer 1 ----
                h1 = actp.tile([128, 8 * SB], BF, tag="h1")
                for m in range(8):
                    ps = psm.tile([128, 512], F32, tag="psm")
                    nc.tensor.matmul(ps[:], w1x_sb[:, m * 128:(m + 1) * 128],
                                     x3t[:, b * SB:(b + 1) * SB],
                                     start=True, stop=False)
                    for kt in range(2):
                        zpk = zp1 if kt == 0 else zp2
                        for v in range(4):
                            kv = kt * 4 + v
                            nc.tensor.matmul(
                                ps[:],
                                wz_sb[:, kv * 1024 + m * 128:kv * 1024 + (m + 1) * 128],
                                zpk[:, v * SB:(v + 1) * SB],
                                start=False, stop=(kt == 1 and v == 3))
                    nc.scalar.activation(h1[:, m * SB:(m + 1) * SB], ps[:],
                                         AF.Relu, bias=b1_sb[:, m:m + 1])
                # ---- layers 2..5 ----
                h2 = actp.tile([128, 8 * SB], BF, tag="h2")
                for m in range(8):
                    ps = psm.tile([128, 512], F32, tag="psm")
                    for kt in range(8):
                        nc.tensor.matmul(
                            ps[:], w2_sb[:, kt * 1024 + m * 128:kt * 1024 + (m + 1) * 128],
                            h1[:, kt * SB:(kt + 1) * SB],
                            start=(kt == 0), stop=(kt == 7))
                    nc.scalar.activation(h2[:, m * SB:(m + 1) * SB], ps[:],
                                         AF.Relu, bias=b2_sb[:, m:m + 1])
                h3 = actp.tile([128, 4 * SB], BF, tag="h1")
                for m in range(4):
                    ps = psm.tile([128, 512], F32, tag="psm")
                    for kt in range(8):
                        nc.tensor.matmul(
                            ps[:], w3_sb[:, kt * 512 + m * 128:kt * 512 + (m + 1) * 128],
                            h2[:, kt * SB:(kt + 1) * SB],
                            start=(kt == 0), stop=(kt == 7))
                    nc.scalar.activation(h3[:, m * SB:(m + 1) * SB], ps[:],
                                         AF.Relu, bias=b3_sb[:, m:m + 1])
                h4 = actp.tile([128, 2 * SB], BF, tag="h2")
                for m in range(2):
                    ps = psm.tile([128, 512], F32, tag="psm")
                    for kt in range(4):
                        nc.tensor.matmul(
                            ps[:], w4_sb[:, kt * 256 + m * 128:kt * 256 + (m + 1) * 128],
                            h3[:, kt * SB:(kt + 1) * SB],
                            start=(kt == 0), stop=(kt == 3))
                    nc.scalar.activation(h4[:, m * SB:(m + 1) * SB], ps[:],
                                         AF.Relu, bias=b4_sb[:, m:m + 1])
                ps5 = psm.tile([1, 512], F32, tag="psm")
                for kt in range(2):
                    nc.tensor.matmul(ps5[:],
                                     w5_sb[:, kt:kt + 1],
                                     h4[:, kt * SB:(kt + 1) * SB],
                                     start=(kt == 0), stop=(kt == 1))
                ob = outp.tile([1, 512], F32, tag="ob")
                nc.scalar.activation(ob[:], ps5[:], AF.Sigmoid,
                                     bias=b5_sb[0:1, 0:1])
                nc.sync.dma_start(out[:, b * SB:(b + 1) * SB], ob[:])
            blk_stack.close()

    nc.compile()
    return nc


def _bf(x):
    return np.asarray(x, np.float32).astype(ml_dtypes.bfloat16)


def _prep_shared(emb_tables, bot_weights, top_weights):
    emb = np.asarray(emb_tables, np.float32)
    tbls = _bf(emb.reshape(T * NROWS, D))

    (W1b, bb1v), (W2b, bb2v), (W3b, bb3v) = [
        (np.asarray(w, np.float32), np.asarray(b_, np.float32))
        for w, b_ in bot_weights
    ]
    tw = [(np.asarray(w, np.float32), np.asarray(b_, np.float32))
          for w, b_ in top_weights]
    W1, b1v = tw[0]

    W1x = _bf(W1[:, :128].T)                       # [128, 1024]
    W1z = W1[:, 128:]                              # [1024, 351]
    li, lj = np.tril_indices(NF, -1)
    conv = np.where(np.arange(NF) == 0, T, np.arange(NF) - 1)
    Wp = np.zeros((NF, NF, 1024), np.float32)
    fi, fj = conv[li], conv[lj]
    Wp[fi, fj, :] = 0.5 * W1z.T
    Wp[fj, fi, :] = 0.5 * W1z.T
    wzv = np.zeros((2, 4, 128, 1024), np.float32)
    for kt in range(2):
        ng = 4 if kt == 0 else 3
        for v in range(4):
            for g in range(ng):
                i = (0 if kt == 0 else 16) + 4 * g + v
                if i <= T:
                    wzv[kt, v, 32 * g:32 * g + NF, :] = Wp[i]
    wz_arr = _bf(wzv.transpose(2, 0, 1, 3).reshape(128, 8 * 1024))

    def ktile(wT):
        # [K, M] -> [128, (K/128)*M] with K-tile-major columns
        K_, M_ = wT.shape
        return np.ascontiguousarray(
            wT.reshape(K_ // 128, 128, M_).transpose(1, 0, 2).reshape(128, -1))

    def bias_tile(b_, parts):
        return np.ascontiguousarray(
            b_.reshape(-1, 128).T if parts == 128 else b_.reshape(1, 1)
        ).astype(np.float32)

    selv = (np.arange(128)[:, None] // 4 == np.arange(32)[None, :])
    shared = {
        "tbls": tbls,
        "sel": _bf(selv.astype(np.float32)),
        "w1b": _bf(W1b.T), "w2b": _bf(ktile(W2b.T)), "w3b": _bf(ktile(W3b.T)),
        "bb1": bias_tile(bb1v, 128), "bb2": bias_tile(bb2v, 128),
        "bb3": bias_tile(bb3v, 128),
        "w1x": W1x, "wz": wz_arr, "b1": bias_tile(b1v, 128),
        "w2": _bf(ktile(tw[1][0].T)), "b2": bias_tile(tw[1][1], 128),
        "w3": _bf(ktile(tw[2][0].T)), "b3": bias_tile(tw[2][1], 128),
        "w4": _bf(ktile(tw[3][0].T)), "b4": bias_tile(tw[3][1], 128),
        "w5": _bf(ktile(tw[4][0].T)), "b5": np.asarray(tw[4][1], np.float32).reshape(1, 1),
    }
    return shared


def _prep_core_idx(lS_core):
    # lS_core: [T, 2048, L] int64 -> [128, NBLK*T*IW] int16 wrapped
    a = lS_core.reshape(T, NBLK, SB, L).astype(np.int64) - BASE
    a = a.reshape(T, NBLK, REAL).astype(np.int16)
    padded = np.concatenate(
        [a, np.zeros((T, NBLK, PAD), np.int16)], axis=2)     # [T, NBLK, NUM]
    p = padded.transpose(1, 0, 2)                            # [NBLK, T, NUM]
    p = p.reshape(NBLK * T, IW, 16).transpose(2, 0, 1)       # [16, NBLK*T, IW]
    p = p.reshape(16, NBLK * T * IW)
    return np.tile(p, (8, 1))                                # [128, cols]


def kernel(dense_x, lS_i, emb_tables, bot_weights, top_weights):
    dense_x = np.asarray(dense_x, np.float32)
    lS_i = np.asarray(lS_i)

    if "nc" not in _cached:
        _cached["nc"] = _build_program()
    nc = _cached["nc"]

    shared = _prep_shared(emb_tables, bot_weights, top_weights)
    in_maps = []
    for c in range(8):
        sl = slice(c * BL, (c + 1) * BL)
        m = dict(shared)
        m["xt"] = _bf(dense_x[sl].T)
        m["idx"] = _prep_core_idx(np.asarray(lS_i[:, sl, :]))
        in_maps.append(m)

    res = bass_utils.run_bass_kernel_spmd(nc, in_maps, core_ids=list(range(8)))
    out = np.concatenate([r["out"].reshape(BL) for r in res.results])
    return out.reshape(B, 1).astype(np.float32)
